# revision 11
# baseline (speedup 1.0000x reference)
"""Per-channel batched Linear (OD matrix) Trainium2 Bass kernel.

Computes out[b,o,c] = sum_t x[b,t,c] * W[c,o,t] + bias[c,o] for
x [128,48,64,64] -> [128,48,4096], W [4096,48,48], bias [4096,48].

Strategy (8 NeuronCores, channel-parallel, 512 channels/core):
  Host pre-stages bf16 images so every DMA is fully contiguous and the
  PE does 256 big matmuls per core with zero on-chip transposes:
  - x image [98, 256g*128b]: rows 0-47 = t of channels 0-255 (j0),
    row 48 = ones (bias), rows 49-96 = t of channels 256-511 (j1),
    row 97 = ones. Columns g-major so pair p's 128 b-columns are the
    contiguous slice [:, p*128:(p+1)*128].
  - W^T image [98, 256p*128m]: per pair p (channels p and p+256),
    block-diagonal lhsT: rows 0-48 x cols 0-47 = aug W^T of ch p
    (48 t rows + bias row), rows 49-97 x cols 64-111 = aug W^T of
    ch p+256, zeros elsewhere (host-built).
  - Per pair: one matmul out[128m,128b] = lhsT.T @ rhs with K=98,
    M=128 (FWL-eligible), N=128, all APs contiguous.
  - psum groups of 16 pairs -> one [128,2048] f32->bf16 copy
    (DVE/ACT alternating) -> one contiguous [128,2048] bf16 store.
  - Output [128, 256p*128b] un-permuted to [B,T,N,N] f32 on host.
"""

import numpy as np
import ml_dtypes

import concourse.bass as bass  # noqa: F401
import concourse.mybir as mybir
import concourse.tile as tile
from concourse import bacc
from concourse.bass_utils import run_bass_kernel_spmd

B, T, O, N = 128, 48, 48, 64
C = N * N
NCORES = 8
CS = C // NCORES  # 512 channels per core
NPAIR = CS // 2  # 256 channel pairs per core
KR = 2 * (T + 1)  # 98 contraction rows (2 x (48 t + 1 bias))
PG = 16  # pairs per psum group (16*128 f32 cols = 8KB/part = 4 banks)
NG = NPAIR // PG  # 16 groups
MP = 96  # stationary cols per pair (A at 0-47, B at 48-95)
XCOLS = NPAIR * B  # 32768
WCOLS = NPAIR * MP  # 24576

F32 = mybir.dt.float32
BF16 = mybir.dt.bfloat16
BF = ml_dtypes.bfloat16


def _body(tc, nc, x_d, w_d, out_d):
    with (
        tc.tile_pool(name="xbf", bufs=1) as xpool,
        tc.tile_pool(name="wt", bufs=1) as wpool,
        tc.tile_pool(name="outs", bufs=16) as opool,
        tc.tile_pool(name="ps", bufs=2, space="PSUM") as ppool,
    ):
        xbf = xpool.tile([128, XCOLS], BF16)
        wt = wpool.tile([128, WCOLS], BF16)
        # wt is (m,p)-major: col = m*NPAIR + p. A-block data lives in
        # rows 0-48 x cols 0-12287, B-block in rows 49-97 x cols
        # 12288-24575; the two off-diagonal quadrants are memset zeros,
        # so only 2.4MB of real weights cross HBM.
        wt3 = wt[:, :].rearrange("k (m p) -> k m p", p=NPAIR)
        # (memset partition base must be 32-aligned; rows 32-48 of the
        # low half are re-written by the A-block DMAs afterwards)
        nc.vector.memset(wt[0 : KR // 2, WCOLS // 2 : WCOLS], 0.0)
        nc.vector.memset(wt[32:64, 0 : WCOLS // 2], 0.0)
        nc.vector.memset(wt[64:KR, 0 : WCOLS // 2], 0.0)
        # All DMA on the single SWDGE ring, in need order, with <=4KB
        # per-partition descriptors (SDMA engines run ~20GB/s at 4KB vs
        # ~14.4 at 8KB). One x chunk + one wt chunk per 16-pair matmul
        # group; stores issue after all loads (FIFO) so they never delay
        # the load stream.
        # Compact wt loads first (every pair's strided lhsT reads the
        # whole wt range, so wt must be fully resident before MM 0).
        NWC = 6
        wcc = WCOLS // 2 // NWC  # 2048 cols = 4KB/partition
        for ch in range(NWC):
            sl = slice(ch * wcc, (ch + 1) * wcc)
            nc.gpsimd.dma_start(wt[0 : KR // 2, sl], w_d[0 : KR // 2, sl])
            nc.gpsimd.dma_start(
                wt[KR // 2 : KR, WCOLS // 2 + ch * wcc : WCOLS // 2 + (ch + 1) * wcc],
                w_d[KR // 2 : KR, sl],
            )
        NCH = NG  # 16 x chunks = one per matmul group
        cc = XCOLS // NCH  # 2048 cols = 4KB/partition
        for ch in range(NCH):
            nc.gpsimd.dma_start(
                xbf[0:KR, ch * cc : (ch + 1) * cc], x_d[:, ch * cc : (ch + 1) * cc]
            )

        for grp in range(NG):
            pt = ppool.tile([128, PG * B], F32)
            for k in range(PG):
                p = grp * PG + k
                nc.tensor.matmul(
                    pt[0:MP, k * B : (k + 1) * B],
                    lhsT=wt3[0:KR, :, p : p + 1],
                    rhs=xbf[0:KR, p * B : (p + 1) * B],
                    start=True,
                    stop=True,
                    skip_group_check=True,
                )
            outs = opool.tile([96, PG * B], BF16)
            if grp % 2 == 0:
                nc.vector.tensor_copy(outs[:, :], pt[0:96, :])
            else:
                nc.scalar.copy(outs[:, :], pt[0:96, :])
            if grp < 10:
                seng = nc.sync if grp % 2 == 0 else nc.scalar
            else:
                seng = nc.gpsimd
            seng.dma_start(
                out_d[:, grp * PG * B : (grp + 1) * PG * B], outs[:, :]
            )


def build_program(num_devices=NCORES):
    nc = bacc.Bacc(
        "TRN2",
        target_bir_lowering=False,
        debug=False,
        enable_asserts=False,
        num_devices=num_devices,
    )
    x_d = nc.dram_tensor("x", [KR, XCOLS], BF16, kind="ExternalInput").ap()
    w_d = nc.dram_tensor("w", [KR, WCOLS // 2], BF16, kind="ExternalInput").ap()
    out_d = nc.dram_tensor("out", [96, XCOLS], BF16, kind="ExternalOutput").ap()
    with tile.TileContext(nc) as tc:
        _body(tc, nc, x_d, w_d, out_d)
    nc.compile()
    return nc


def _stage_inputs(x, W, bias):
    """Build per-core bf16 x/W images (host-side, not on HW critical path)."""
    xb = np.ascontiguousarray(x, dtype=np.float32).reshape(B, T, C).astype(BF)
    WTt = np.ascontiguousarray(W, dtype=np.float32).transpose(0, 2, 1).astype(BF)
    bb = np.ascontiguousarray(bias, dtype=np.float32).astype(BF)
    in_maps = []
    for i in range(NCORES):
        sl = slice(i * CS, (i + 1) * CS)
        xc = xb[:, :, sl]  # [B, T, 512]
        ximg = np.empty([KR, NPAIR, B], dtype=BF)
        ximg[0:T] = xc[:, :, 0:NPAIR].transpose(1, 2, 0)
        ximg[T] = np.ones([NPAIR, B], dtype=BF)
        ximg[T + 1 : KR - 1] = xc[:, :, NPAIR:CS].transpose(1, 2, 0)
        ximg[KR - 1] = np.ones([NPAIR, B], dtype=BF)
        wc = WTt[sl]  # [512, 48t, 48o]
        bc = bb[sl]  # [512, 48o]
        # (m,p)-major compact image: rows 0-48 = A blocks [49, 48o, 256p],
        # rows 49-97 = B blocks.
        wimg = np.empty([KR, O, NPAIR], dtype=BF)
        wimg[0:T, :, :] = wc[0:NPAIR].transpose(1, 2, 0)
        wimg[T, :, :] = bc[0:NPAIR].T
        wimg[T + 1 : KR - 1, :, :] = wc[NPAIR:CS].transpose(1, 2, 0)
        wimg[KR - 1, :, :] = bc[NPAIR:CS].T
        in_maps.append(
            {
                "x": np.ascontiguousarray(ximg.reshape(KR, XCOLS)),
                "w": np.ascontiguousarray(wimg.reshape(KR, WCOLS // 2)),
            }
        )
    return in_maps


_CACHED_NC = None
LAST_RESULT = None


def kernel(**inputs) -> np.ndarray:
    global _CACHED_NC, LAST_RESULT
    in_maps = _stage_inputs(inputs["x"], inputs["W"], inputs["b"])

    if _CACHED_NC is None:
        _CACHED_NC = build_program(NCORES)
    nc = _CACHED_NC

    res = run_bass_kernel_spmd(nc, in_maps, core_ids=list(range(NCORES)))
    LAST_RESULT = res
    out = np.empty([B, T, C], dtype=np.float32)
    for i in range(NCORES):
        img = np.asarray(res.results[i]["out"]).reshape(96, NPAIR, B)
        sl0 = slice(i * CS, i * CS + NPAIR)
        sl1 = slice(i * CS + NPAIR, (i + 1) * CS)
        # out[b, o, p] = img[o, p, b] (ch p); img[64+o, p, b] (ch p+256)
        out[:, :, sl0] = img[0:O].transpose(2, 0, 1).astype(np.float32)
        out[:, :, sl1] = img[O : 2 * O].transpose(2, 0, 1).astype(np.float32)
    return out.reshape(B, T, N, N)


# revision 12
# speedup vs baseline: 1.3150x; 1.3150x over previous
"""Per-channel batched Linear (OD matrix) Trainium2 Bass kernel.

Computes out[b,o,c] = sum_t x[b,t,c] * W[c,o,t] + bias[c,o] for
x [128,48,64,64] -> [128,48,4096], W [4096,48,48], bias [4096,48].

Strategy (8 NeuronCores, channel-parallel, 512 channels/core):
  Host pre-stages bf16 images so every DMA is fully contiguous and the
  PE does 256 big matmuls per core with zero on-chip transposes:
  - x image [98, 256g*128b]: rows 0-47 = t of channels 0-255 (j0),
    row 48 = ones (bias), rows 49-96 = t of channels 256-511 (j1),
    row 97 = ones. Columns g-major so pair p's 128 b-columns are the
    contiguous slice [:, p*128:(p+1)*128].
  - W^T image [98, 256p*128m]: per pair p (channels p and p+256),
    block-diagonal lhsT: rows 0-48 x cols 0-47 = aug W^T of ch p
    (48 t rows + bias row), rows 49-97 x cols 64-111 = aug W^T of
    ch p+256, zeros elsewhere (host-built).
  - Per pair: one matmul out[128m,128b] = lhsT.T @ rhs with K=98,
    M=128 (FWL-eligible), N=128, all APs contiguous.
  - psum groups of 16 pairs -> one [128,2048] f32->bf16 copy
    (DVE/ACT alternating) -> one contiguous [128,2048] bf16 store.
  - Output [128, 256p*128b] un-permuted to [B,T,N,N] f32 on host.
"""

import numpy as np
import ml_dtypes

import concourse.bass as bass  # noqa: F401
import concourse.mybir as mybir
import concourse.tile as tile
from concourse import bacc
from concourse.bass_utils import run_bass_kernel_spmd

B, T, O, N = 128, 48, 48, 64
C = N * N
NCORES = 8
CS = C // NCORES  # 512 channels per core
NPAIR = CS // 2  # 256 channel pairs per core
KR = 2 * (T + 1)  # 98 contraction rows (2 x (48 t + 1 bias))
PG = 16  # pairs per psum group (16*128 f32 cols = 8KB/part = 4 banks)
NG = NPAIR // PG  # 16 groups
MP = 96  # stationary cols per pair (A at 0-47, B at 48-95)
XCOLS = NPAIR * B  # 32768
WCOLS = NPAIR * MP  # 24576

F32 = mybir.dt.float32
BF16 = mybir.dt.bfloat16
BF = ml_dtypes.bfloat16


def _body(tc, nc, x_d, w_d, out_d):
    with (
        tc.tile_pool(name="xbf", bufs=1) as xpool,
        tc.tile_pool(name="wt", bufs=1) as wpool,
        tc.tile_pool(name="outs", bufs=16) as opool,
        tc.tile_pool(name="ps", bufs=2, space="PSUM") as ppool,
    ):
        xbf = xpool.tile([128, XCOLS], BF16)
        wt = wpool.tile([128, WCOLS], BF16)
        # All loads on the single SWDGE ring in need order with <=4KB
        # per-partition descriptors; one x chunk + one wt chunk per
        # 16-pair matmul group.
        NCH = NG
        cc = XCOLS // NCH  # 2048 cols = 4KB/partition
        wcc = WCOLS // NCH  # 1536 cols = 3KB/partition
        for ch in range(NCH):
            nc.gpsimd.dma_start(
                xbf[0:KR, ch * cc : (ch + 1) * cc], x_d[:, ch * cc : (ch + 1) * cc]
            )
            nc.gpsimd.dma_start(
                wt[0:KR, ch * wcc : (ch + 1) * wcc], w_d[:, ch * wcc : (ch + 1) * wcc]
            )

        for grp in range(NG):
            pt = ppool.tile([128, PG * B], F32)
            for k in range(PG):
                p = grp * PG + k
                nc.tensor.matmul(
                    pt[0:MP, k * B : (k + 1) * B],
                    lhsT=wt[0:KR, p * MP : (p + 1) * MP],
                    rhs=xbf[0:KR, p * B : (p + 1) * B],
                    start=True,
                    stop=True,
                    skip_group_check=True,
                )
            outs = opool.tile([96, PG * B], BF16)
            if grp % 2 == 0:
                nc.vector.tensor_copy(outs[:, :], pt[0:96, :])
            else:
                nc.scalar.copy(outs[:, :], pt[0:96, :])
            if grp < 10:
                seng = nc.sync if grp % 2 == 0 else nc.scalar
            else:
                seng = nc.gpsimd
            seng.dma_start(
                out_d[:, grp * PG * B : (grp + 1) * PG * B], outs[:, :]
            )


def build_program(num_devices=NCORES):
    nc = bacc.Bacc(
        "TRN2",
        target_bir_lowering=False,
        debug=False,
        enable_asserts=False,
        num_devices=num_devices,
    )
    x_d = nc.dram_tensor("x", [KR, XCOLS], BF16, kind="ExternalInput").ap()
    w_d = nc.dram_tensor("w", [KR, WCOLS], BF16, kind="ExternalInput").ap()
    out_d = nc.dram_tensor("out", [96, XCOLS], BF16, kind="ExternalOutput").ap()
    with tile.TileContext(nc) as tc:
        _body(tc, nc, x_d, w_d, out_d)
    nc.compile()
    return nc


def _stage_inputs(x, W, bias):
    """Build per-core bf16 x/W images (host-side, not on HW critical path)."""
    xb = np.ascontiguousarray(x, dtype=np.float32).reshape(B, T, C).astype(BF)
    WTt = np.ascontiguousarray(W, dtype=np.float32).transpose(0, 2, 1).astype(BF)
    bb = np.ascontiguousarray(bias, dtype=np.float32).astype(BF)
    in_maps = []
    for i in range(NCORES):
        sl = slice(i * CS, (i + 1) * CS)
        xc = xb[:, :, sl]  # [B, T, 512]
        ximg = np.empty([KR, NPAIR, B], dtype=BF)
        ximg[0:T] = xc[:, :, 0:NPAIR].transpose(1, 2, 0)
        ximg[T] = np.ones([NPAIR, B], dtype=BF)
        ximg[T + 1 : KR - 1] = xc[:, :, NPAIR:CS].transpose(1, 2, 0)
        ximg[KR - 1] = np.ones([NPAIR, B], dtype=BF)
        wc = WTt[sl]  # [512, 48t, 48o]
        bc = bb[sl]  # [512, 48o]
        wimg = np.zeros([KR, NPAIR, MP], dtype=BF)
        wimg[0:T, :, 0:O] = wc[0:NPAIR].transpose(1, 0, 2)
        wimg[T, :, 0:O] = bc[0:NPAIR]
        wimg[T + 1 : KR - 1, :, O : 2 * O] = wc[NPAIR:CS].transpose(1, 0, 2)
        wimg[KR - 1, :, O : 2 * O] = bc[NPAIR:CS]
        in_maps.append(
            {
                "x": np.ascontiguousarray(ximg.reshape(KR, XCOLS)),
                "w": np.ascontiguousarray(wimg.reshape(KR, WCOLS)),
            }
        )
    return in_maps


_CACHED_NC = None
LAST_RESULT = None


def kernel(**inputs) -> np.ndarray:
    global _CACHED_NC, LAST_RESULT
    in_maps = _stage_inputs(inputs["x"], inputs["W"], inputs["b"])

    if _CACHED_NC is None:
        _CACHED_NC = build_program(NCORES)
    nc = _CACHED_NC

    res = run_bass_kernel_spmd(nc, in_maps, core_ids=list(range(NCORES)))
    LAST_RESULT = res
    out = np.empty([B, T, C], dtype=np.float32)
    for i in range(NCORES):
        img = np.asarray(res.results[i]["out"]).reshape(96, NPAIR, B)
        sl0 = slice(i * CS, i * CS + NPAIR)
        sl1 = slice(i * CS + NPAIR, (i + 1) * CS)
        # out[b, o, p] = img[o, p, b] (ch p); img[64+o, p, b] (ch p+256)
        out[:, :, sl0] = img[0:O].transpose(2, 0, 1).astype(np.float32)
        out[:, :, sl1] = img[O : 2 * O].transpose(2, 0, 1).astype(np.float32)
    return out.reshape(B, T, N, N)
